# revision 1
# baseline (speedup 1.0000x reference)
"""InternLM3 attention block on 8 Trainium2 NeuronCores (Bass/Tile).

Strategy (tensor-parallel over heads, per the GQA structure):
  - 32 Q heads / 8 KV heads, head_dim 128.  Core c owns Q heads [4c,4c+4)
    and KV head c (one GQA group per core, so K/V never needs replication).
  - Per core, fused pipeline over 512-token blocks: QKV projection (fp32r
    matmuls, transposed layout [dim, tok]) -> RoPE (DVE) -> causal
    flash-style attention in S^T orientation (scores^T = k^T-tiles.T @ q^T,
    exp on ACT, PV accumulated in PSUM, denominator via DVE partial sums +
    one ones-matmul, normalize with gpsimd partition_broadcast).
  - Attention outputs (attn^T, [512 hid-slice, tok]) are AllGathered across
    the 8 cores in 8 token-chunks (overlapped with compute), then each core
    computes its 512-column slice of the output projection.
  - Host only shards/transposes inputs and concatenates the 8 output slices.

All matmuls run as float32r (TF32-like, full PE rate, ~1e-4 relative error).
"""

import math
import os
import sys

if "/opt/trn_rl_repo" not in sys.path:
    sys.path.insert(0, "/opt/trn_rl_repo")

import numpy as np

import concourse.bass as bass
import concourse.mybir as mybir
import concourse.tile as tile
from concourse import bacc
from concourse import bass_utils

# ---- problem constants (hardcoded per harness contract) ----
HIDDEN = 4096
N_HEADS = 32
N_KV_HEADS = 8
HEAD_DIM = 128
ROPE_THETA = 10000.0
B, S = 2, 2048
NCORES = 8

P = 128
TQ = 512                      # token block
NB = S // TQ                  # 4 blocks per batch
KT = HIDDEN // P              # 32 contraction tiles
QH = N_HEADS // NCORES        # 4 q-heads per core
HG = QH * HEAD_DIM            # 512 = head-group width per core
NCHUNK = B * NB               # 8 allgather chunks
TOK = B * S                   # 4096 tokens

f32 = mybir.dt.float32
f32r = mybir.dt.float32r
_PHASES = "all"  # all | qkv | noproj (skip outproj)


def _build_module(with_collectives=True):
    nc = bacc.Bacc("TRN2", target_bir_lowering=False, debug=False,
                   num_devices=NCORES)
    nc._skip_collectives = not with_collectives

    xT = nc.dram_tensor("xT", [HIDDEN, TOK], f32r, kind="ExternalInput").ap()
    wqT = nc.dram_tensor("wqT", [HIDDEN, HG], f32r, kind="ExternalInput").ap()
    wkT = nc.dram_tensor("wkT", [HIDDEN, HEAD_DIM], f32r, kind="ExternalInput").ap()
    wvT = nc.dram_tensor("wvT", [HIDDEN, HEAD_DIM], f32r, kind="ExternalInput").ap()
    woT = nc.dram_tensor("woT", [HIDDEN, HG], f32r, kind="ExternalInput").ap()
    cosT = nc.dram_tensor("cosT", [P, S], f32, kind="ExternalInput").ap()
    ssinT = nc.dram_tensor("ssinT", [P, S], f32, kind="ExternalInput").ap()
    masksIn = nc.dram_tensor("masksIn", [P, 4 * TQ], f32, kind="ExternalInput").ap()
    identIn = nc.dram_tensor("identIn", [P, P], f32, kind="ExternalInput").ap()
    onesIn = nc.dram_tensor("onesIn", [P, 1], f32r, kind="ExternalInput").ap()
    outT = nc.dram_tensor("outT", [HG, TOK], f32, kind="ExternalOutput").ap()

    ag_in = [
        nc.dram_tensor(f"ag_in{i}", [HG, TQ], f32r, kind="Internal").ap()
        for i in range(NCHUNK)
    ]
    ag_out = [
        nc.dram_tensor(f"ag_out{i}", [HIDDEN, TQ], f32r, kind="Internal",
                       addr_space="Shared").ap()
        for i in range(NCHUNK)
    ]

    with tile.TileContext(nc) as tc:
        _body(tc, nc, xT, wqT, wkT, wvT, woT, cosT, ssinT, masksIn, identIn,
              onesIn, outT, ag_in, ag_out)
    nc.compile()
    return nc


def _body(tc, nc, xT, wqT, wkT, wvT, woT, cosT, ssinT, masksIn, identIn,
          onesIn, outT, ag_in, ag_out):
    AF = mybir.ActivationFunctionType
    OP = mybir.AluOpType

    with (
        tc.tile_pool(name="wpool", bufs=1) as wpool,
        tc.tile_pool(name="xpool", bufs=2) as xpool,
        tc.tile_pool(name="kvpool", bufs=1) as kvpool,
        tc.tile_pool(name="qpool", bufs=1) as qpool,
        tc.tile_pool(name="stage", bufs=1) as stage,
        tc.tile_pool(name="epool", bufs=4) as epool,
        tc.tile_pool(name="aux", bufs=2) as aux,
        tc.tile_pool(name="pq", bufs=1, space="PSUM") as pq,
        tc.tile_pool(name="ppv", bufs=1, space="PSUM") as ppv,
        tc.tile_pool(name="pst", bufs=2, space="PSUM") as pst,
    ):
        # ---- resident constants / weights ----
        wq_sb = wpool.tile([P, KT, HG], f32r, tag="wq")
        nc.sync.dma_start(wq_sb[:], wqT.rearrange("(ko p) m -> p ko m", p=P))
        wk_sb = wpool.tile([P, KT, HEAD_DIM], f32r, tag="wk")
        nc.sync.dma_start(wk_sb[:], wkT.rearrange("(ko p) m -> p ko m", p=P))
        wv_sb = wpool.tile([P, KT, HEAD_DIM], f32r, tag="wv")
        nc.sync.dma_start(wv_sb[:], wvT.rearrange("(ko p) m -> p ko m", p=P))
        cos_sb = wpool.tile([P, S], f32, tag="cos")
        nc.sync.dma_start(cos_sb[:], cosT)
        sin_sb = wpool.tile([P, S], f32, tag="sin")
        nc.sync.dma_start(sin_sb[:], ssinT)
        mask_sb = wpool.tile([P, 4, TQ], f32, tag="mask")
        nc.sync.dma_start(mask_sb[:], masksIn.rearrange("p (r t) -> p r t", r=4))
        id_sb = wpool.tile([P, P], f32, tag="ident")
        nc.sync.dma_start(id_sb[:], identIn)
        ones_sb = wpool.tile([P, 1], f32r, tag="ones")
        nc.sync.dma_start(ones_sb[:], onesIn)

        def rope(dst_f32r, src_sb, n):
            """dst = src*cos + rotate_half(src)*sin for token block n.

            dst is a [P, TQ] f32r AP, src_sb a [P, TQ] fp32 SBUF AP."""
            cos_blk = cos_sb[:, n * TQ:(n + 1) * TQ]
            sin_blk = sin_sb[:, n * TQ:(n + 1) * TQ]
            rt = stage.tile([P, TQ], f32, tag="ropetmp")
            nc.vector.tensor_copy(rt[0:64, :], src_sb[64:128, :])
            nc.vector.tensor_copy(rt[64:128, :], src_sb[0:64, :])
            nc.vector.tensor_tensor(rt[:], rt[:], sin_blk, OP.mult)
            nc.vector.tensor_tensor(dst_f32r, src_sb, cos_blk, OP.mult)
            nc.vector.tensor_tensor(
                dst_f32r, dst_f32r.bitcast(f32), rt[:], OP.add
            )

        for b in range(B):
            kT_cache = kvpool.tile([P, S], f32r, tag="kT")
            v_cache = kvpool.tile([P, S // P, HEAD_DIM], f32r, tag="v")
            for n in range(NB):
                tok0 = b * S + n * TQ
                # ---------- QKV projection for this token block ----------
                q_ps = [
                    pq.tile([P, TQ], f32, tag=f"q{j}", name=f"qps{j}")
                    for j in range(QH)
                ]
                k_ps = pq.tile([P, TQ], f32, tag="kk")
                v_ps = ppv.tile([P, TQ], f32, tag="pv")
                KB = 4  # k-tiles per 1MB DMA batch
                for k4 in range(KT // KB):
                    x_t = xpool.tile([P, KB, TQ], f32r, tag="x")
                    nc.sync.dma_start(
                        x_t[:],
                        xT[k4 * KB * P:(k4 + 1) * KB * P,
                           tok0:tok0 + TQ].rearrange(
                            "(ko p) t -> p ko t", p=P),
                    )
                    for kk in range(KB):
                        k = k4 * KB + kk
                        st = dict(start=(k == 0), stop=(k == KT - 1))
                        for j in range(QH):
                            nc.tensor.matmul(
                                q_ps[j][:], wq_sb[:, k, j * P:(j + 1) * P],
                                x_t[:, kk, :], **st
                            )
                        nc.tensor.matmul(
                            k_ps[:], wk_sb[:, k, :], x_t[:, kk, :], **st)
                        nc.tensor.matmul(
                            v_ps[:], wv_sb[:, k, :], x_t[:, kk, :], **st)

                # free PSUM banks fast: ACT copies PSUM -> SBUF staging
                qstage = [
                    stage.tile([P, TQ], f32, tag=f"qs{j}", name=f"qstage{j}")
                    for j in range(QH)
                ]
                kstage = stage.tile([P, TQ], f32, tag="ks")
                vT_sb = stage.tile([P, TQ], f32, tag="vtsb")
                for j in range(QH):
                    nc.scalar.copy(qstage[j][:], q_ps[j][:])
                nc.scalar.copy(kstage[:], k_ps[:])
                nc.vector.tensor_copy(vT_sb[:], v_ps[:])

                # ---------- RoPE ----------
                qT_sb = qpool.tile([P, QH, TQ], f32r, tag="q")
                for j in range(QH):
                    rope(qT_sb[:, j, :], qstage[j][:], n)
                rope(kT_cache[:, n * TQ:(n + 1) * TQ], kstage[:], n)
                for j in range(4):
                    tp = pst.tile([P, TQ], f32, tag="st")
                    nc.tensor.transpose(
                        tp[:, :P], vT_sb[:, j * P:(j + 1) * P], id_sb[:]
                    )
                    nc.vector.tensor_copy(
                        v_cache[:, n * 4 + j, :], tp[:, :P]
                    )

                # ---------- attention, one GQA head at a time ----------
                ntk = (n + 1) * (TQ // P)
                for h in range(QH if _PHASES != "qkv" else 0):
                    acc = aux.tile([P, TQ], f32r, tag="acc")
                    pv_ps = ppv.tile([P, TQ], f32, tag="pv")
                    qr = qT_sb[:, h, :]
                    for t in range(ntk):
                        st_ps = pst.tile([P, TQ], f32, tag="st")
                        nc.tensor.matmul(
                            st_ps[:], kT_cache[:, t * P:(t + 1) * P], qr,
                            start=True, stop=True,
                        )
                        es = epool.tile([P, TQ], f32r, tag="es")
                        nc.scalar.activation(es[:], st_ps[:], AF.Exp)
                        if t >= ntk - 4:
                            nc.vector.tensor_tensor(
                                es[:], es[:].bitcast(f32),
                                mask_sb[:, t - (ntk - 4), :], OP.mult,
                            )
                        if t == 0:
                            nc.vector.tensor_copy(acc[:], es[:].bitcast(f32))
                        else:
                            nc.vector.tensor_tensor(
                                acc[:], acc[:].bitcast(f32),
                                es[:].bitcast(f32), OP.add,
                            )
                        nc.tensor.matmul(
                            pv_ps[:], v_cache[:, t, :], es[:],
                            start=(t == 0), stop=(t == ntk - 1),
                        )
                    # denominator + normalize
                    dn_ps = pst.tile([P, TQ], f32, tag="st")
                    nc.tensor.matmul(
                        dn_ps[:1, :], ones_sb[:], acc[:],
                        start=True, stop=True,
                    )
                    rec = aux.tile([1, TQ], f32, tag="rec")
                    nc.vector.reciprocal(rec[:], dn_ps[:1, :])
                    bc = aux.tile([P, TQ], f32, tag="bc")
                    nc.gpsimd.partition_broadcast(bc[:], rec[:])
                    ao = aux.tile([P, TQ], f32r, tag="ao")
                    nc.vector.tensor_tensor(ao[:], pv_ps[:], bc[:], OP.mult)
                    ch = b * NB + n
                    nc.sync.dma_start(
                        ag_in[ch][h * P:(h + 1) * P, :], ao[:].bitcast(f32r)
                    )

                # ---------- AllGather this chunk across the 8 cores ----------
                ch = b * NB + n
                if not getattr(nc, "_skip_collectives", False):
                    nc.gpsimd.collective_compute(
                        "AllGather",
                        mybir.AluOpType.bypass,
                        replica_groups=[list(range(NCORES))],
                        ins=[ag_in[ch].opt()],
                        outs=[ag_out[ch].opt()],
                    )

    # ---------- output projection: out[:, c*512:(c+1)*512] ----------
    TO = 256  # token sub-block
    with (
        tc.tile_pool(name="wopool", bufs=1) as wopool,
        tc.tile_pool(name="atpool", bufs=2) as atpool,
        tc.tile_pool(name="obpool", bufs=3) as obpool,
        tc.tile_pool(name="pop", bufs=4, space="PSUM") as pop,
    ):
        wo_sb = wopool.tile([P, KT, HG], f32r, tag="wo")
        nc.sync.dma_start(wo_sb[:], woT.rearrange("(ko p) m -> p ko m", p=P))
        for ch in range(NCHUNK if _PHASES == "all" else 0):
            ag_r = ag_out[ch].rearrange("(ko p) t -> p ko t", p=P)
            for half in range(TQ // TO):
                at = atpool.tile([P, KT, TO], f32r, tag="at")
                for qtr in range(4):
                    nc.sync.dma_start(
                        at[:, qtr * 8:(qtr + 1) * 8, :],
                        ag_r[:, qtr * 8:(qtr + 1) * 8,
                             half * TO:(half + 1) * TO],
                    )
                for m in range(HG // P):
                    op_ps = pop.tile([P, TO], f32, tag="op")
                    for k in range(KT):
                        nc.tensor.matmul(
                            op_ps[:], wo_sb[:, k, m * P:(m + 1) * P],
                            at[:, k, :],
                            start=(k == 0), stop=(k == KT - 1),
                        )
                    ob = obpool.tile([P, TO], f32, tag="ob")
                    nc.vector.tensor_copy(ob[:], op_ps[:])
                    c0 = ch * TQ + half * TO
                    nc.sync.dma_start(
                        outT[m * P:(m + 1) * P, c0:c0 + TO], ob[:]
                    )


_NC_CACHE = None


def _get_module():
    global _NC_CACHE
    if _NC_CACHE is None:
        _NC_CACHE = _build_module()
    return _NC_CACHE


def _host_consts():
    inv_freq = 1.0 / (ROPE_THETA ** (np.arange(0, HEAD_DIM, 2,
                                               dtype=np.float32) / HEAD_DIM))
    t = np.arange(S, dtype=np.float32)
    freqs = np.outer(t, inv_freq).astype(np.float32)      # [S, 64]
    cos_h = np.cos(freqs).T                               # [64, S]
    sin_h = np.sin(freqs).T
    cosT = np.concatenate([cos_h, cos_h], axis=0).astype(np.float32)
    ssinT = np.concatenate([-sin_h, sin_h], axis=0).astype(np.float32)

    i = np.arange(P)[:, None]
    j = np.arange(TQ)[None, :]
    masks = np.concatenate(
        [(i + r * P <= j).astype(np.float32) for r in range(4)], axis=1
    )                                                     # [128, 4*512]
    ident = np.eye(P, dtype=np.float32)
    ones = np.ones((P, 1), dtype=np.float32)
    return cosT, ssinT, masks, ident, ones


def make_in_maps(hidden_states, wq, wk, wv, wo):
    hidden_states = np.asarray(hidden_states, dtype=np.float32)
    wq = np.asarray(wq, dtype=np.float32)
    wk = np.asarray(wk, dtype=np.float32)
    wv = np.asarray(wv, dtype=np.float32)
    wo = np.asarray(wo, dtype=np.float32)

    xT = np.ascontiguousarray(hidden_states.reshape(TOK, HIDDEN).T)
    cosT, ssinT, masks, ident, ones = _host_consts()
    qscale = 1.0 / math.sqrt(HEAD_DIM)

    in_maps = []
    for c in range(NCORES):
        in_maps.append({
            "xT": xT,
            "wqT": np.ascontiguousarray(
                (wq[c * HG:(c + 1) * HG] * qscale).T),
            "wkT": np.ascontiguousarray(
                wk[c * HEAD_DIM:(c + 1) * HEAD_DIM].T),
            "wvT": np.ascontiguousarray(
                wv[c * HEAD_DIM:(c + 1) * HEAD_DIM].T),
            "woT": np.ascontiguousarray(wo[c * HG:(c + 1) * HG].T),
            "cosT": cosT,
            "ssinT": ssinT,
            "masksIn": masks,
            "identIn": ident,
            "onesIn": ones,
        })
    return in_maps


def assemble_output(results):
    out = np.empty((TOK, HIDDEN), dtype=np.float32)
    for c in range(NCORES):
        out[:, c * HG:(c + 1) * HG] = results[c]["outT"].T
    return out.reshape(B, S, HIDDEN)


def kernel(hidden_states, wq, wk, wv, wo):
    nc = _get_module()
    in_maps = make_in_maps(hidden_states, wq, wk, wv, wo)
    trace = bool(int(os.environ.get("KERNEL_TRACE", "0")))
    res = bass_utils.run_bass_kernel_spmd(
        nc, in_maps, core_ids=list(range(NCORES)), trace=trace
    )
    if trace:
        kernel.last_results = res
    return assemble_output(res.results)


kernel.last_results = None



# revision 2
# speedup vs baseline: 1.2990x; 1.2990x over previous
"""InternLM3 attention block on 8 Trainium2 NeuronCores (Bass/Tile), v2.

Strategy (tensor-parallel over heads, per the GQA structure):
  - 32 Q heads / 8 KV heads, head_dim 128.  Core c owns Q heads [4c,4c+4)
    and KV head c (one GQA group per core, so K/V never needs replication).
  - All matmul operands bf16 (inputs cast on host): same PE rate as fp32r
    but half the HBM/SBUF traffic and 2x DVE throughput.  PSUM accum f32.
  - Per core, per 512-token block: QKV projection (V produced directly in
    [tok, d] layout via 128-wide sub-matmuls, no transposes) -> RoPE on
    DVE/Pool (bf16, in place) -> causal flash attention in S^T orientation,
    two heads per pass: scores^T for both heads land in one 2-bank PSUM
    tile (double-buffered), one batched exp (ACT) per k-tile, softmax
    denominator accumulated per-head on DVE/Pool, PV accumulated in PSUM
    with the PV matmul one k-step behind the score matmul so PE never
    waits on ACT.
  - Attention outputs are AllGathered across the 8 cores per 512-token
    chunk (bf16); the output projection for chunk i-1 is interleaved into
    block i's attention t-loop to fill PE slack, remainder drains in a
    dense tail.  Each core computes its 512-column slice of out.
"""

import math
import os
import sys

if "/opt/trn_rl_repo" not in sys.path:
    sys.path.insert(0, "/opt/trn_rl_repo")

import ml_dtypes
import numpy as np

import concourse.bass as bass
import concourse.mybir as mybir
import concourse.tile as tile
from concourse import bacc
from concourse import bass_utils

# ---- problem constants (hardcoded per harness contract) ----
HIDDEN = 4096
N_HEADS = 32
N_KV_HEADS = 8
HEAD_DIM = 128
ROPE_THETA = 10000.0
B, S = 2, 2048
NCORES = 8

P = 128
TQ = 512                      # token block
NB = S // TQ                  # 4 blocks per batch
KT = HIDDEN // P              # 32 contraction tiles
KB = 8                        # k-tiles per x DMA batch
QH = N_HEADS // NCORES        # 4 q-heads per core
HG = QH * HEAD_DIM            # 512 = head-group width per core
NCHUNK = B * NB               # 8 allgather chunks
TOK = B * S                   # 4096 tokens
TO = 256                      # outproj token sub-block

f32 = mybir.dt.float32
bf16 = mybir.dt.bfloat16
npbf16 = ml_dtypes.bfloat16


def _build_module(with_collectives=True):
    nc = bacc.Bacc("TRN2", target_bir_lowering=False, debug=False,
                   num_devices=NCORES)
    nc._skip_collectives = not with_collectives

    xT = nc.dram_tensor("xT", [HIDDEN, TOK], bf16, kind="ExternalInput").ap()
    wqT = nc.dram_tensor("wqT", [HIDDEN, HG], bf16, kind="ExternalInput").ap()
    wkT = nc.dram_tensor("wkT", [HIDDEN, HEAD_DIM], bf16, kind="ExternalInput").ap()
    wvT = nc.dram_tensor("wvT", [HIDDEN, HEAD_DIM], bf16, kind="ExternalInput").ap()
    woT = nc.dram_tensor("woT", [HIDDEN, HG], bf16, kind="ExternalInput").ap()
    cosT = nc.dram_tensor("cosT", [P, S], bf16, kind="ExternalInput").ap()
    ssinT = nc.dram_tensor("ssinT", [P, S], bf16, kind="ExternalInput").ap()
    masksIn = nc.dram_tensor("masksIn", [P, 4 * TQ], bf16, kind="ExternalInput").ap()
    onesIn = nc.dram_tensor("onesIn", [P, 1], bf16, kind="ExternalInput").ap()
    outT = nc.dram_tensor("outT", [HG, TOK], f32, kind="ExternalOutput").ap()

    ag_in = [
        nc.dram_tensor(f"ag_in{i}", [HG, TQ], bf16, kind="Internal").ap()
        for i in range(NCHUNK)
    ]
    ag_out = [
        nc.dram_tensor(f"ag_out{i}", [HIDDEN, TQ], bf16, kind="Internal",
                       addr_space="Shared").ap()
        for i in range(NCHUNK)
    ]

    with tile.TileContext(nc) as tc:
        _body(tc, nc, xT, wqT, wkT, wvT, woT, cosT, ssinT, masksIn,
              onesIn, outT, ag_in, ag_out)
    nc.compile()
    return nc


class OutprojEmitter:
    """Queue of output-projection emission closures, drained op-by-op into
    PE slack inside the attention t-loops (remainder drains densely)."""

    def __init__(self, nc, wo_sb, atpool, obpool, psum, outT, ag_out):
        self.nc = nc
        self.wo_sb = wo_sb
        self.atpool = atpool
        self.obpool = obpool
        self.psum = psum
        self.outT = outT
        self.ag_out = ag_out
        self.q = []
        self.ob_flip = 0

    def add_chunk(self, ch):
        nc = self.nc
        KC = 8  # matmuls per emitted closure
        ats = {}

        def load(half):
            at = self.atpool.tile([P, KT, TO], bf16, tag="at", bufs=2,
                                  name="at")
            nc.sync.dma_start(
                at[:],
                self.ag_out[ch].rearrange("(ko p) t -> p ko t", p=P)[
                    :, :, half * TO:(half + 1) * TO],
            )
            ats[half] = at

        for half in range(TQ // TO):
            self.q.append(lambda half=half: load(half))
        for half in range(TQ // TO):
            for m in range(HG // P):
                def start_chain(half=half, m=m):
                    self._op_ps = self.psum.tile([P, TO], f32, tag="pop",
                                                 bufs=2, name="op_ps")

                def mms(half=half, m=m, k0=0, first=False, last=False):
                    if first:
                        start_chain(half, m)
                    op_ps, at = self._op_ps, ats[half]
                    for k in range(k0, k0 + KC):
                        nc.tensor.matmul(
                            op_ps[:], self.wo_sb[:, k, m * P:(m + 1) * P],
                            at[:, k, :],
                            start=(k == 0), stop=(k == KT - 1),
                        )
                    if last:
                        ob = self.obpool.tile([P, TO], f32, tag="ob", bufs=3,
                                              name="ob")
                        nc.vector.tensor_copy(ob[:], op_ps[:])
                        c0 = ch * TQ + half * TO
                        nc.sync.dma_start(
                            self.outT[m * P:(m + 1) * P, c0:c0 + TO], ob[:])

                for k0 in range(0, KT, KC):
                    self.q.append(
                        lambda half=half, m=m, k0=k0: mms(
                            half, m, k0, first=(k0 == 0),
                            last=(k0 + KC == KT)))

    def emit(self, n):
        while n > 0 and self.q:
            self.q.pop(0)()
            n -= 1

    def drain(self):
        self.emit(len(self.q))


def _body(tc, nc, xT, wqT, wkT, wvT, woT, cosT, ssinT, masksIn,
          onesIn, outT, ag_in, ag_out):
    AF = mybir.ActivationFunctionType
    OP = mybir.AluOpType

    with (
        tc.tile_pool(name="wpool", bufs=1) as wpool,
        tc.tile_pool(name="xpool", bufs=2) as xpool,
        tc.tile_pool(name="kvpool", bufs=1) as kvpool,
        tc.tile_pool(name="qpool", bufs=1) as qpool,
        tc.tile_pool(name="epool", bufs=3) as epool,
        tc.tile_pool(name="accpool", bufs=1) as accpool,
        tc.tile_pool(name="aux", bufs=2) as aux,
        tc.tile_pool(name="atpool", bufs=2) as atpool,
        tc.tile_pool(name="obpool", bufs=3) as obpool,
        tc.tile_pool(name="psum", bufs=1, space="PSUM") as psum,
    ):
        # ---- resident constants / weights.  QKV weights + x go on the SP
        # DMA queue interleaved (block 0 starts after ~2MB); everything not
        # needed immediately goes on the ACT DMA queue in parallel.
        wq_sb = wpool.tile([P, KT, HG], bf16, tag="wq")
        wk_sb = wpool.tile([P, KT, HEAD_DIM], bf16, tag="wk")
        wv_sb = wpool.tile([P, KT, HEAD_DIM], bf16, tag="wv")

        def load_wq_chunk(c4):
            nc.sync.dma_start(
                wq_sb[:, c4 * 4:(c4 + 1) * 4, :],
                wqT[c4 * 4 * P:(c4 + 1) * 4 * P, :].rearrange(
                    "(ko p) m -> p ko m", p=P))

        load_wq_chunk(0)
        # block-0 x batches are interleaved with the remaining wq chunks and
        # wk/wv inside the first QKV loop (deferred_w), so PE starts after
        # ~1.5MB of DMA instead of the full weight set.
        deferred_w = [
            lambda: nc.sync.dma_start(
                wk_sb[:], wkT.rearrange("(ko p) m -> p ko m", p=P)),
            lambda: nc.sync.dma_start(
                wv_sb[:], wvT.rearrange("(ko p) m -> p ko m", p=P)),
        ] + [lambda c4=c4: load_wq_chunk(c4) for c4 in range(1, 8)]
        cos_sb = wpool.tile([P, S], bf16, tag="cos")
        nc.scalar.dma_start(cos_sb[:], cosT)
        sin_sb = wpool.tile([P, S], bf16, tag="sin")
        nc.scalar.dma_start(sin_sb[:], ssinT)
        mask_sb = wpool.tile([P, 4, TQ], bf16, tag="mask")
        nc.scalar.dma_start(mask_sb[:], masksIn.rearrange("p (r t) -> p r t", r=4))
        ones_sb = wpool.tile([P, 1], bf16, tag="ones")
        nc.scalar.dma_start(ones_sb[:], onesIn)
        wo_sb = wpool.tile([P, KT, HG], bf16, tag="wo")
        nc.scalar.dma_start(wo_sb[:], woT.rearrange("(ko p) m -> p ko m", p=P))

        emitter = OutprojEmitter(nc, wo_sb, atpool, obpool, psum, outT, ag_out)

        def rope(eng, tmptag, dst, n):
            """In-place RoPE on bf16 SBUF tile dst [P, TQ] for block n."""
            cos_blk = cos_sb[:, n * TQ:(n + 1) * TQ]
            sin_blk = sin_sb[:, n * TQ:(n + 1) * TQ]
            rt = aux.tile([P, TQ], bf16, tag=tmptag, bufs=2, name="ropetmp")
            eng.tensor_copy(rt[0:64, :], dst[64:128, :])
            eng.tensor_copy(rt[64:128, :], dst[0:64, :])
            eng.tensor_tensor(rt[:], rt[:], sin_blk, OP.mult)
            eng.tensor_tensor(dst, dst, cos_blk, OP.mult)
            eng.tensor_tensor(dst, dst, rt[:], OP.add)

        def normalize(h, pv_ps, acc, ch):
            """softmax denominator + divide for head h, ship to ag_in."""
            dn_ps = psum.tile([1, TQ], f32, tag="pop", bufs=2, name="dn_ps")
            nc.tensor.matmul(dn_ps[:1, :], ones_sb[:], acc[:],
                             start=True, stop=True)
            rec = aux.tile([1, TQ], f32, tag="rec", name="rec")
            nc.vector.reciprocal(rec[:], dn_ps[:1, :])
            bc = aux.tile([P, TQ], f32, tag="bc", name="bc")
            nc.gpsimd.partition_broadcast(bc[:], rec[:])
            ao = aux.tile([P, TQ], bf16, tag="ao", name="ao")
            nc.vector.tensor_tensor(ao[:], pv_ps, bc[:], OP.mult)
            nc.sync.dma_start(ag_in[ch][h * P:(h + 1) * P, :], ao[:])

        for b in range(B):
            kT_cache = kvpool.tile([P, S], bf16, tag="kT")
            v_cache = kvpool.tile([P, S // P, HEAD_DIM], bf16, tag="v")
            for n in range(NB):
                i_blk = b * NB + n
                ch = i_blk
                tok0 = b * S + n * TQ
                ntk = (n + 1) * (TQ // P)

                # ---------- QKV projection for this token block ----------
                # The whole x block stays resident (two 16-ktile tiles), so
                # the six output chains run sequentially (PSUM accumulation
                # groups are per-bank) and each chain's PSUM drain + RoPE
                # overlaps the later chains.  By the time the k/v chains
                # finish, q0/q1 are already roped, so pass A starts with no
                # boundary stall.
                xa = xpool.tile([P, KT // 2, TQ], bf16, tag="xa", bufs=1, name="xa")
                xb = xpool.tile([P, KT // 2, TQ], bf16, tag="xb", bufs=1, name="xb")
                for xi, xt_ in ((0, xa), (1, xb)):
                    for hh in range(2):
                        ko0 = xi * 16 + hh * 8
                        nc.sync.dma_start(
                            xt_[:, hh * 8:(hh + 1) * 8, :],
                            xT[ko0 * P:(ko0 + 8) * P,
                               tok0:tok0 + TQ].rearrange(
                                "(ko p) t -> p ko t", p=P),
                        )
                        if i_blk == 0:
                            for _ in range(3 if xi == 0 and hh == 0 else 2):
                                if deferred_w:
                                    deferred_w.pop(0)()
                if i_blk >= 1:
                    # at-tile loads for the chunk whose outproj is
                    # interleaved into this block (its AllGather was issued
                    # at the end of the previous block)
                    emitter.add_chunk(i_blk - 1)
                    emitter.emit(2)  # the two at-load DMAs

                def xk(k):
                    return (xa if k < 16 else xb)[:, k % 16, :]

                qT_sb = qpool.tile([P, QH, TQ], bf16, tag="q", name="qT_sb")
                kblk = kT_cache[:, n * TQ:(n + 1) * TQ]

                q01 = psum.tile([P, 2, TQ], f32, tag="qA", name="q01")
                for j in range(2):
                    for k in range(KT):
                        nc.tensor.matmul(
                            q01[:, j, :], wq_sb[:, k, j * P:(j + 1) * P],
                            xk(k), start=(k == 0), stop=(k == KT - 1))
                nc.scalar.copy(qT_sb[:, 0:2, :], q01[:])
                rope(nc.vector, "rtD", qT_sb[:, 0, :], n)
                rope(nc.vector, "rtD", qT_sb[:, 1, :], n)

                q23 = psum.tile([P, 2, TQ], f32, tag="qB", name="q23")
                for j in range(2):
                    for k in range(KT):
                        nc.tensor.matmul(
                            q23[:, j, :], wq_sb[:, k, (j + 2) * P:(j + 3) * P],
                            xk(k), start=(k == 0), stop=(k == KT - 1))
                nc.scalar.copy(qT_sb[:, 2:4, :], q23[:])
                rope(nc.vector, "rtD", qT_sb[:, 2, :], n)
                rope(nc.vector, "rtD", qT_sb[:, 3, :], n)

                k_ps = psum.tile([P, TQ], f32, tag="kk", name="k_ps")
                for k in range(KT):
                    nc.tensor.matmul(k_ps[:], wk_sb[:, k, :], xk(k),
                                     start=(k == 0), stop=(k == KT - 1))
                nc.scalar.copy(kblk, k_ps[:])
                rope(nc.vector, "rtD", kblk, n)

                v4 = psum.tile([P, 4, HEAD_DIM], f32, tag="vv", name="v4")
                for j in range(4):
                    for k in range(KT):
                        nc.tensor.matmul(
                            v4[:, j, :], xk(k)[:, j * P:(j + 1) * P],
                            wv_sb[:, k, :], start=(k == 0), stop=(k == KT - 1))
                nc.scalar.copy(
                    v_cache[:, n * 4:(n + 1) * 4, :], v4[:])

                # ---------- attention: two heads per pass ----------
                for pas in range(2):
                    h0, h1 = 2 * pas, 2 * pas + 1
                    pv0 = psum.tile([P, TQ], f32, tag="kk", name="pv0")
                    pv1 = psum.tile([P, TQ], f32, tag="vv", name="pv1")
                    acc0 = accpool.tile([P, TQ], bf16, tag=f"acc{h0}",
                                        name="acc0")
                    acc1 = accpool.tile([P, TQ], bf16, tag=f"acc{h1}",
                                        name="acc1")
                    es_prev = None
                    for t in range(ntk):
                        emitter.emit(1)
                        st2 = psum.tile([P, 2, TQ], f32,
                                        tag=("qA" if t % 2 == 0 else "qB"),
                                        name="st2")
                        kt = kT_cache[:, t * P:(t + 1) * P]
                        nc.tensor.matmul(st2[:, 0, :], kt, qT_sb[:, h0, :],
                                         start=True, stop=True)
                        nc.tensor.matmul(st2[:, 1, :], kt, qT_sb[:, h1, :],
                                         start=True, stop=True)
                        if es_prev is not None:
                            tp = t - 1
                            nc.tensor.matmul(
                                pv0[:], v_cache[:, tp, :], es_prev[:, 0, :],
                                start=(tp == 0), stop=False)
                            nc.tensor.matmul(
                                pv1[:], v_cache[:, tp, :], es_prev[:, 1, :],
                                start=(tp == 0), stop=False)
                        es2 = epool.tile([P, 2, TQ], bf16, tag="es",
                                         name="es2")
                        nc.scalar.activation(
                            es2.rearrange("p h t -> p (h t)"),
                            st2.rearrange("p h t -> p (h t)"), AF.Exp)
                        r = t - (ntk - 4)
                        if r >= 0:
                            mk = mask_sb[:, r, :]
                            nc.vector.tensor_tensor(
                                es2[:, 0, :], es2[:, 0, :], mk, OP.mult)
                            nc.vector.tensor_tensor(
                                es2[:, 1, :], es2[:, 1, :], mk, OP.mult)
                        if t == 0:
                            nc.vector.tensor_copy(acc0[:], es2[:, 0, :])
                            nc.gpsimd.tensor_copy(acc1[:], es2[:, 1, :])
                        else:
                            nc.vector.tensor_tensor(
                                acc0[:], acc0[:], es2[:, 0, :], OP.add)
                            nc.gpsimd.tensor_tensor(
                                acc1[:], acc1[:], es2[:, 1, :], OP.add)
                        es_prev = es2
                    tp = ntk - 1
                    nc.tensor.matmul(pv0[:], v_cache[:, tp, :],
                                     es_prev[:, 0, :],
                                     start=(tp == 0), stop=True)
                    nc.tensor.matmul(pv1[:], v_cache[:, tp, :],
                                     es_prev[:, 1, :],
                                     start=(tp == 0), stop=True)
                    normalize(h0, pv0[:], acc0, ch)
                    normalize(h1, pv1[:], acc1, ch)

                # ---------- AllGather this chunk across the 8 cores ----
                if not getattr(nc, "_skip_collectives", False):
                    nc.gpsimd.collective_compute(
                        "AllGather",
                        mybir.AluOpType.bypass,
                        replica_groups=[list(range(NCORES))],
                        ins=[ag_in[ch].opt()],
                        outs=[ag_out[ch].opt()],
                    )

        # ---------- tail: remaining output projection ----------
        emitter.add_chunk(NCHUNK - 1)
        emitter.drain()


_NC_CACHE = None


def _get_module():
    global _NC_CACHE
    if _NC_CACHE is None:
        _NC_CACHE = _build_module(
            with_collectives=not bool(int(os.environ.get("KERNEL_NO_CC", "0"))))
    return _NC_CACHE


def _host_consts():
    inv_freq = 1.0 / (ROPE_THETA ** (np.arange(0, HEAD_DIM, 2,
                                               dtype=np.float32) / HEAD_DIM))
    t = np.arange(S, dtype=np.float32)
    freqs = np.outer(t, inv_freq).astype(np.float32)      # [S, 64]
    cos_h = np.cos(freqs).T                               # [64, S]
    sin_h = np.sin(freqs).T
    cosT = np.concatenate([cos_h, cos_h], axis=0)
    ssinT = np.concatenate([-sin_h, sin_h], axis=0)

    i = np.arange(P)[:, None]
    j = np.arange(TQ)[None, :]
    masks = np.concatenate(
        [(i + r * P <= j).astype(np.float32) for r in range(4)], axis=1
    )                                                     # [128, 4*512]
    ones = np.ones((P, 1), dtype=np.float32)
    return (cosT.astype(npbf16), ssinT.astype(npbf16),
            masks.astype(npbf16), ones.astype(npbf16))


def make_in_maps(hidden_states, wq, wk, wv, wo):
    hidden_states = np.asarray(hidden_states, dtype=np.float32)
    wq = np.asarray(wq, dtype=np.float32)
    wk = np.asarray(wk, dtype=np.float32)
    wv = np.asarray(wv, dtype=np.float32)
    wo = np.asarray(wo, dtype=np.float32)

    xT = np.ascontiguousarray(
        hidden_states.reshape(TOK, HIDDEN).T).astype(npbf16)
    cosT, ssinT, masks, ones = _host_consts()
    qscale = 1.0 / math.sqrt(HEAD_DIM)

    in_maps = []
    for c in range(NCORES):
        in_maps.append({
            "xT": xT,
            "wqT": np.ascontiguousarray(
                (wq[c * HG:(c + 1) * HG] * qscale).T).astype(npbf16),
            "wkT": np.ascontiguousarray(
                wk[c * HEAD_DIM:(c + 1) * HEAD_DIM].T).astype(npbf16),
            "wvT": np.ascontiguousarray(
                wv[c * HEAD_DIM:(c + 1) * HEAD_DIM].T).astype(npbf16),
            "woT": np.ascontiguousarray(wo[c * HG:(c + 1) * HG].T).astype(npbf16),
            "cosT": cosT,
            "ssinT": ssinT,
            "masksIn": masks,
            "onesIn": ones,
        })
    return in_maps


def assemble_output(results):
    out = np.empty((TOK, HIDDEN), dtype=np.float32)
    for c in range(NCORES):
        out[:, c * HG:(c + 1) * HG] = results[c]["outT"].T
    return out.reshape(B, S, HIDDEN)


def kernel(hidden_states, wq, wk, wv, wo):
    nc = _get_module()
    in_maps = make_in_maps(hidden_states, wq, wk, wv, wo)
    trace = bool(int(os.environ.get("KERNEL_TRACE", "0")))
    res = bass_utils.run_bass_kernel_spmd(
        nc, in_maps, core_ids=list(range(NCORES)), trace=trace
    )
    if trace:
        kernel.last_results = res
    return assemble_output(res.results)


kernel.last_results = None


# revision 3
# speedup vs baseline: 1.3448x; 1.0353x over previous
"""InternLM3 attention block on 8 Trainium2 NeuronCores (Bass/Tile), v2.

Strategy (tensor-parallel over heads, per the GQA structure):
  - 32 Q heads / 8 KV heads, head_dim 128.  Core c owns Q heads [4c,4c+4)
    and KV head c (one GQA group per core, so K/V never needs replication).
  - All matmul operands bf16 (inputs cast on host): same PE rate as fp32r
    but half the HBM/SBUF traffic and 2x DVE throughput.  PSUM accum f32.
  - Per core, per 512-token block: QKV projection (V produced directly in
    [tok, d] layout via 128-wide sub-matmuls, no transposes) -> RoPE on
    DVE/Pool (bf16, in place) -> causal flash attention in S^T orientation,
    two heads per pass: scores^T for both heads land in one 2-bank PSUM
    tile (double-buffered), one batched exp (ACT) per k-tile, softmax
    denominator accumulated per-head on DVE/Pool, PV accumulated in PSUM
    with the PV matmul one k-step behind the score matmul so PE never
    waits on ACT.
  - Attention outputs are AllGathered across the 8 cores per 512-token
    chunk (bf16); the output projection for chunk i-1 is interleaved into
    block i's attention t-loop to fill PE slack, remainder drains in a
    dense tail.  Each core computes its 512-column slice of out.
"""

import math
import os
import sys

if "/opt/trn_rl_repo" not in sys.path:
    sys.path.insert(0, "/opt/trn_rl_repo")

import ml_dtypes
import numpy as np

import concourse.bass as bass
import concourse.mybir as mybir
import concourse.tile as tile
from concourse import bacc
from concourse import bass_utils

# ---- problem constants (hardcoded per harness contract) ----
HIDDEN = 4096
N_HEADS = 32
N_KV_HEADS = 8
HEAD_DIM = 128
ROPE_THETA = 10000.0
B, S = 2, 2048
NCORES = 8

P = 128
TQ = 512                      # token block
NB = S // TQ                  # 4 blocks per batch
KT = HIDDEN // P              # 32 contraction tiles
KB = 8                        # k-tiles per x DMA batch
QH = N_HEADS // NCORES        # 4 q-heads per core
HG = QH * HEAD_DIM            # 512 = head-group width per core
NCHUNK = B * NB               # 8 allgather chunks
TOK = B * S                   # 4096 tokens
TO = 256                      # outproj token sub-block

f32 = mybir.dt.float32
bf16 = mybir.dt.bfloat16
npbf16 = ml_dtypes.bfloat16


def _build_module(with_collectives=True):
    nc = bacc.Bacc("TRN2", target_bir_lowering=False, debug=False,
                   num_devices=NCORES)
    nc._skip_collectives = not with_collectives

    xT = nc.dram_tensor("xT", [HIDDEN, TOK], bf16, kind="ExternalInput").ap()
    wqkvo = nc.dram_tensor("wqkvo", [HIDDEN, 2 * HG + 2 * HEAD_DIM], bf16,
                           kind="ExternalInput").ap()
    tables = nc.dram_tensor("tables", [P, 2 * S + 4 * TQ + 1], bf16,
                            kind="ExternalInput").ap()
    wqT = wqkvo[:, 0:HG]
    wkT = wqkvo[:, HG:HG + HEAD_DIM]
    wvT = wqkvo[:, HG + HEAD_DIM:HG + 2 * HEAD_DIM]
    woT = wqkvo[:, HG + 2 * HEAD_DIM:]
    cosT = tables[:, 0:S]
    ssinT = tables[:, S:2 * S]
    masksIn = tables[:, 2 * S:2 * S + 4 * TQ]
    onesIn = tables[:, 2 * S + 4 * TQ:]
    outT = nc.dram_tensor("outT", [HG, TOK], f32, kind="ExternalOutput").ap()

    ag_in = [
        nc.dram_tensor(f"ag_in{i}", [HG, TQ], bf16, kind="Internal").ap()
        for i in range(NCHUNK)
    ]
    ag_out = [
        nc.dram_tensor(f"ag_out{i}", [HIDDEN, TQ], bf16, kind="Internal",
                       addr_space="Shared").ap()
        for i in range(NCHUNK)
    ]

    with tile.TileContext(nc) as tc:
        _body(tc, nc, xT, wqT, wkT, wvT, woT, cosT, ssinT, masksIn,
              onesIn, outT, ag_in, ag_out)
    nc.compile()
    return nc


class OutprojEmitter:
    """Queue of output-projection emission closures, drained op-by-op into
    PE slack inside the attention t-loops (remainder drains densely)."""

    def __init__(self, nc, wo_sb, atpool, obpool, psum, outT, ag_out):
        self.nc = nc
        self.wo_sb = wo_sb
        self.atpool = atpool
        self.obpool = obpool
        self.psum = psum
        self.outT = outT
        self.ag_out = ag_out
        self.q = []
        self.ob_flip = 0

    def add_chunk(self, ch):
        nc = self.nc
        KC = 8  # matmuls per emitted closure
        ats = {}

        def load(half):
            at = self.atpool.tile([P, KT, TO], bf16, tag="at", bufs=2,
                                  name="at")
            nc.sync.dma_start(
                at[:],
                self.ag_out[ch].rearrange("(ko p) t -> p ko t", p=P)[
                    :, :, half * TO:(half + 1) * TO],
            )
            ats[half] = at

        for half in range(TQ // TO):
            self.q.append(lambda half=half: load(half))
        for half in range(TQ // TO):
            for m in range(HG // P):
                def start_chain(half=half, m=m):
                    self._op_ps = self.psum.tile([P, TO], f32, tag="pop",
                                                 bufs=2, name="op_ps")

                def mms(half=half, m=m, k0=0, first=False, last=False):
                    if first:
                        start_chain(half, m)
                    op_ps, at = self._op_ps, ats[half]
                    for k in range(k0, k0 + KC):
                        nc.tensor.matmul(
                            op_ps[:], self.wo_sb[:, k, m * P:(m + 1) * P],
                            at[:, k, :],
                            start=(k == 0), stop=(k == KT - 1),
                        )
                    if last:
                        ob = self.obpool.tile([P, TO], f32, tag="ob", bufs=3,
                                              name="ob")
                        nc.vector.tensor_copy(ob[:], op_ps[:])
                        c0 = ch * TQ + half * TO
                        nc.sync.dma_start(
                            self.outT[m * P:(m + 1) * P, c0:c0 + TO], ob[:])

                for k0 in range(0, KT, KC):
                    self.q.append(
                        lambda half=half, m=m, k0=k0: mms(
                            half, m, k0, first=(k0 == 0),
                            last=(k0 + KC == KT)))

    def emit(self, n):
        while n > 0 and self.q:
            self.q.pop(0)()
            n -= 1

    def drain(self):
        self.emit(len(self.q))


def _body(tc, nc, xT, wqT, wkT, wvT, woT, cosT, ssinT, masksIn,
          onesIn, outT, ag_in, ag_out):
    AF = mybir.ActivationFunctionType
    OP = mybir.AluOpType

    with (
        tc.tile_pool(name="wpool", bufs=1) as wpool,
        tc.tile_pool(name="xpool", bufs=2) as xpool,
        tc.tile_pool(name="kvpool", bufs=1) as kvpool,
        tc.tile_pool(name="qpool", bufs=1) as qpool,
        tc.tile_pool(name="epool", bufs=3) as epool,
        tc.tile_pool(name="accpool", bufs=1) as accpool,
        tc.tile_pool(name="aux", bufs=2) as aux,
        tc.tile_pool(name="atpool", bufs=2) as atpool,
        tc.tile_pool(name="obpool", bufs=3) as obpool,
        tc.tile_pool(name="psum", bufs=1, space="PSUM") as psum,
    ):
        # ---- resident constants / weights.  QKV weights + x go on the SP
        # DMA queue interleaved (block 0 starts after ~2MB); everything not
        # needed immediately goes on the ACT DMA queue in parallel.
        wq_sb = wpool.tile([P, KT, HG], bf16, tag="wq")
        wk_sb = wpool.tile([P, KT, HEAD_DIM], bf16, tag="wk")
        wv_sb = wpool.tile([P, KT, HEAD_DIM], bf16, tag="wv")

        def load_wq_chunk(c4):
            nc.sync.dma_start(
                wq_sb[:, c4 * 4:(c4 + 1) * 4, :],
                wqT[c4 * 4 * P:(c4 + 1) * 4 * P, :].rearrange(
                    "(ko p) m -> p ko m", p=P))

        load_wq_chunk(0)
        # block-0 x batches are interleaved with the remaining wq chunks and
        # wk/wv inside the first QKV loop (deferred_w), so PE starts after
        # ~1.5MB of DMA instead of the full weight set.
        deferred_w = [
            lambda: nc.sync.dma_start(
                wk_sb[:], wkT.rearrange("(ko p) m -> p ko m", p=P)),
            lambda: nc.sync.dma_start(
                wv_sb[:], wvT.rearrange("(ko p) m -> p ko m", p=P)),
        ] + [lambda c4=c4: load_wq_chunk(c4) for c4 in range(1, 8)]
        cos_sb = wpool.tile([P, S], bf16, tag="cos")
        nc.scalar.dma_start(cos_sb[:], cosT)
        sin_sb = wpool.tile([P, S], bf16, tag="sin")
        nc.scalar.dma_start(sin_sb[:], ssinT)
        mask_sb = wpool.tile([P, 4, TQ], bf16, tag="mask")
        nc.scalar.dma_start(mask_sb[:], masksIn.rearrange("p (r t) -> p r t", r=4))
        ones_sb = wpool.tile([P, 1], bf16, tag="ones")
        nc.scalar.dma_start(ones_sb[:], onesIn)
        wo_sb = wpool.tile([P, KT, HG], bf16, tag="wo")
        nc.scalar.dma_start(wo_sb[:], woT.rearrange("(ko p) m -> p ko m", p=P))

        emitter = OutprojEmitter(nc, wo_sb, atpool, obpool, psum, outT, ag_out)

        def rope(eng, tmptag, dst, n):
            """In-place RoPE on bf16 SBUF tile dst [P, TQ] for block n."""
            cos_blk = cos_sb[:, n * TQ:(n + 1) * TQ]
            sin_blk = sin_sb[:, n * TQ:(n + 1) * TQ]
            rt = aux.tile([P, TQ], bf16, tag=tmptag, bufs=2, name="ropetmp")
            eng.tensor_copy(rt[0:64, :], dst[64:128, :])
            eng.tensor_copy(rt[64:128, :], dst[0:64, :])
            eng.tensor_tensor(rt[:], rt[:], sin_blk, OP.mult)
            eng.tensor_tensor(dst, dst, cos_blk, OP.mult)
            eng.tensor_tensor(dst, dst, rt[:], OP.add)

        def normalize(h, pv_ps, acc, ch):
            """softmax denominator + divide for head h, ship to ag_in."""
            dn_ps = psum.tile([1, TQ], f32, tag="pop", bufs=2, name="dn_ps")
            nc.tensor.matmul(dn_ps[:1, :], ones_sb[:], acc[:],
                             start=True, stop=True)
            rec = aux.tile([1, TQ], f32, tag="rec", name="rec")
            nc.vector.reciprocal(rec[:], dn_ps[:1, :])
            bc = aux.tile([P, TQ], f32, tag="bc", name="bc")
            nc.gpsimd.partition_broadcast(bc[:], rec[:])
            ao = aux.tile([P, TQ], bf16, tag="ao", name="ao")
            nc.vector.tensor_tensor(ao[:], pv_ps, bc[:], OP.mult)
            nc.sync.dma_start(ag_in[ch][h * P:(h + 1) * P, :], ao[:])

        for b in range(B):
            kT_cache = kvpool.tile([P, S], bf16, tag="kT")
            v_cache = kvpool.tile([P, S // P, HEAD_DIM], bf16, tag="v")
            for n in range(NB):
                i_blk = b * NB + n
                ch = i_blk
                tok0 = b * S + n * TQ
                ntk = (n + 1) * (TQ // P)

                # ---------- QKV projection for this token block ----------
                # The whole x block stays resident (two 16-ktile tiles), so
                # the six output chains run sequentially (PSUM accumulation
                # groups are per-bank) and each chain's PSUM drain + RoPE
                # overlaps the later chains.  By the time the k/v chains
                # finish, q0/q1 are already roped, so pass A starts with no
                # boundary stall.
                xa = xpool.tile([P, KT // 2, TQ], bf16, tag="xa", bufs=1, name="xa")
                xb = xpool.tile([P, KT // 2, TQ], bf16, tag="xb", bufs=1, name="xb")
                for xi, xt_ in ((0, xa), (1, xb)):
                    for hh in range(2):
                        ko0 = xi * 16 + hh * 8
                        nc.sync.dma_start(
                            xt_[:, hh * 8:(hh + 1) * 8, :],
                            xT[ko0 * P:(ko0 + 8) * P,
                               tok0:tok0 + TQ].rearrange(
                                "(ko p) t -> p ko t", p=P),
                        )
                        if i_blk == 0:
                            for _ in range(3 if xi == 0 and hh == 0 else 2):
                                if deferred_w:
                                    deferred_w.pop(0)()
                if i_blk >= 1:
                    # at-tile loads for the chunk whose outproj is
                    # interleaved into this block (its AllGather was issued
                    # at the end of the previous block)
                    emitter.add_chunk(i_blk - 1)
                    emitter.emit(2)  # the two at-load DMAs

                def xk(k):
                    return (xa if k < 16 else xb)[:, k % 16, :]

                qT_sb = qpool.tile([P, QH, TQ], bf16, tag="q", name="qT_sb")
                kblk = kT_cache[:, n * TQ:(n + 1) * TQ]

                q01 = psum.tile([P, 2, TQ], f32, tag="qA", name="q01")
                for j in range(2):
                    for k in range(KT):
                        nc.tensor.matmul(
                            q01[:, j, :], wq_sb[:, k, j * P:(j + 1) * P],
                            xk(k), start=(k == 0), stop=(k == KT - 1))
                nc.scalar.copy(qT_sb[:, 0:2, :], q01[:])
                rope(nc.vector, "rtD", qT_sb[:, 0, :], n)
                rope(nc.vector, "rtD", qT_sb[:, 1, :], n)

                q23 = psum.tile([P, 2, TQ], f32, tag="qB", name="q23")
                for j in range(2):
                    for k in range(KT):
                        nc.tensor.matmul(
                            q23[:, j, :], wq_sb[:, k, (j + 2) * P:(j + 3) * P],
                            xk(k), start=(k == 0), stop=(k == KT - 1))
                nc.scalar.copy(qT_sb[:, 2:4, :], q23[:])
                rope(nc.vector, "rtD", qT_sb[:, 2, :], n)
                rope(nc.vector, "rtD", qT_sb[:, 3, :], n)

                k_ps = psum.tile([P, TQ], f32, tag="kk", name="k_ps")
                for k in range(KT):
                    nc.tensor.matmul(k_ps[:], wk_sb[:, k, :], xk(k),
                                     start=(k == 0), stop=(k == KT - 1))
                nc.scalar.copy(kblk, k_ps[:])
                rope(nc.vector, "rtD", kblk, n)

                v4 = psum.tile([P, 4, HEAD_DIM], f32, tag="vv", name="v4")
                for j in range(4):
                    for k in range(KT):
                        nc.tensor.matmul(
                            v4[:, j, :], xk(k)[:, j * P:(j + 1) * P],
                            wv_sb[:, k, :], start=(k == 0), stop=(k == KT - 1))
                nc.scalar.copy(
                    v_cache[:, n * 4:(n + 1) * 4, :], v4[:])

                # ---------- attention: two heads per pass ----------
                for pas in range(2):
                    h0, h1 = 2 * pas, 2 * pas + 1
                    pv0 = psum.tile([P, TQ], f32, tag="kk", name="pv0")
                    pv1 = psum.tile([P, TQ], f32, tag="vv", name="pv1")
                    acc0 = accpool.tile([P, TQ], bf16, tag=f"acc{h0}",
                                        name="acc0")
                    acc1 = accpool.tile([P, TQ], bf16, tag=f"acc{h1}",
                                        name="acc1")
                    es_prev = None
                    for t in range(ntk):
                        emitter.emit(1)
                        st2 = psum.tile([P, 2, TQ], f32,
                                        tag=("qA" if t % 2 == 0 else "qB"),
                                        name="st2")
                        kt = kT_cache[:, t * P:(t + 1) * P]
                        nc.tensor.matmul(st2[:, 0, :], kt, qT_sb[:, h0, :],
                                         start=True, stop=True)
                        nc.tensor.matmul(st2[:, 1, :], kt, qT_sb[:, h1, :],
                                         start=True, stop=True)
                        if es_prev is not None:
                            tp = t - 1
                            nc.tensor.matmul(
                                pv0[:], v_cache[:, tp, :], es_prev[:, 0, :],
                                start=(tp == 0), stop=False)
                            nc.tensor.matmul(
                                pv1[:], v_cache[:, tp, :], es_prev[:, 1, :],
                                start=(tp == 0), stop=False)
                        es2 = epool.tile([P, 2, TQ], bf16, tag="es",
                                         name="es2")
                        nc.scalar.activation(
                            es2.rearrange("p h t -> p (h t)"),
                            st2.rearrange("p h t -> p (h t)"), AF.Exp)
                        r = t - (ntk - 4)
                        if r >= 0:
                            mk = mask_sb[:, r, :]
                            nc.vector.tensor_tensor(
                                es2[:, 0, :], es2[:, 0, :], mk, OP.mult)
                            nc.vector.tensor_tensor(
                                es2[:, 1, :], es2[:, 1, :], mk, OP.mult)
                        if t == 0:
                            nc.vector.tensor_copy(acc0[:], es2[:, 0, :])
                            nc.gpsimd.tensor_copy(acc1[:], es2[:, 1, :])
                        else:
                            nc.vector.tensor_tensor(
                                acc0[:], acc0[:], es2[:, 0, :], OP.add)
                            nc.gpsimd.tensor_tensor(
                                acc1[:], acc1[:], es2[:, 1, :], OP.add)
                        es_prev = es2
                    tp = ntk - 1
                    nc.tensor.matmul(pv0[:], v_cache[:, tp, :],
                                     es_prev[:, 0, :],
                                     start=(tp == 0), stop=True)
                    nc.tensor.matmul(pv1[:], v_cache[:, tp, :],
                                     es_prev[:, 1, :],
                                     start=(tp == 0), stop=True)
                    normalize(h0, pv0[:], acc0, ch)
                    normalize(h1, pv1[:], acc1, ch)

                # ---------- AllGather this chunk across the 8 cores ----
                if not getattr(nc, "_skip_collectives", False):
                    nc.gpsimd.collective_compute(
                        "AllGather",
                        mybir.AluOpType.bypass,
                        replica_groups=[list(range(NCORES))],
                        ins=[ag_in[ch].opt()],
                        outs=[ag_out[ch].opt()],
                    )

        # ---------- tail: remaining output projection ----------
        emitter.add_chunk(NCHUNK - 1)
        emitter.drain()


_NC_CACHE = None


def _get_module():
    global _NC_CACHE
    if _NC_CACHE is None:
        _NC_CACHE = _build_module(
            with_collectives=not bool(int(os.environ.get("KERNEL_NO_CC", "0"))))
    return _NC_CACHE


def _host_consts():
    inv_freq = 1.0 / (ROPE_THETA ** (np.arange(0, HEAD_DIM, 2,
                                               dtype=np.float32) / HEAD_DIM))
    t = np.arange(S, dtype=np.float32)
    freqs = np.outer(t, inv_freq).astype(np.float32)      # [S, 64]
    cos_h = np.cos(freqs).T                               # [64, S]
    sin_h = np.sin(freqs).T
    cosT = np.concatenate([cos_h, cos_h], axis=0)
    ssinT = np.concatenate([-sin_h, sin_h], axis=0)

    i = np.arange(P)[:, None]
    j = np.arange(TQ)[None, :]
    masks = np.concatenate(
        [(i + r * P <= j).astype(np.float32) for r in range(4)], axis=1
    )                                                     # [128, 4*512]
    ones = np.ones((P, 1), dtype=np.float32)
    return (cosT.astype(npbf16), ssinT.astype(npbf16),
            masks.astype(npbf16), ones.astype(npbf16))


def make_in_maps(hidden_states, wq, wk, wv, wo):
    hidden_states = np.asarray(hidden_states, dtype=np.float32)
    wq = np.asarray(wq, dtype=np.float32)
    wk = np.asarray(wk, dtype=np.float32)
    wv = np.asarray(wv, dtype=np.float32)
    wo = np.asarray(wo, dtype=np.float32)

    xT = np.ascontiguousarray(
        hidden_states.reshape(TOK, HIDDEN).T).astype(npbf16)
    cosT, ssinT, masks, ones = _host_consts()
    qscale = 1.0 / math.sqrt(HEAD_DIM)
    tables = np.concatenate(
        [cosT, ssinT, masks, ones], axis=1).astype(npbf16)

    in_maps = []
    for c in range(NCORES):
        wqkvo = np.concatenate([
            (wq[c * HG:(c + 1) * HG] * qscale).T,
            wk[c * HEAD_DIM:(c + 1) * HEAD_DIM].T,
            wv[c * HEAD_DIM:(c + 1) * HEAD_DIM].T,
            wo[c * HG:(c + 1) * HG].T,
        ], axis=1).astype(npbf16)
        in_maps.append({
            "xT": xT,
            "wqkvo": np.ascontiguousarray(wqkvo),
            "tables": tables,
        })
    return in_maps


def assemble_output(results):
    out = np.empty((TOK, HIDDEN), dtype=np.float32)
    for c in range(NCORES):
        out[:, c * HG:(c + 1) * HG] = results[c]["outT"].T
    return out.reshape(B, S, HIDDEN)


def kernel(hidden_states, wq, wk, wv, wo):
    nc = _get_module()
    in_maps = make_in_maps(hidden_states, wq, wk, wv, wo)
    trace = bool(int(os.environ.get("KERNEL_TRACE", "0")))
    res = bass_utils.run_bass_kernel_spmd(
        nc, in_maps, core_ids=list(range(NCORES)), trace=trace
    )
    if trace:
        kernel.last_results = res
    return assemble_output(res.results)


kernel.last_results = None


# revision 5
# speedup vs baseline: 1.3750x; 1.0225x over previous
"""InternLM3 attention block on 8 Trainium2 NeuronCores (Bass/Tile), v2.

Strategy (tensor-parallel over heads, per the GQA structure):
  - 32 Q heads / 8 KV heads, head_dim 128.  Core c owns Q heads [4c,4c+4)
    and KV head c (one GQA group per core, so K/V never needs replication).
  - All matmul operands bf16 (inputs cast on host): same PE rate as fp32r
    but half the HBM/SBUF traffic and 2x DVE throughput.  PSUM accum f32.
  - Per core, per 512-token block: QKV projection (V produced directly in
    [tok, d] layout via 128-wide sub-matmuls, no transposes) -> RoPE on
    DVE/Pool (bf16, in place) -> causal flash attention in S^T orientation,
    two heads per pass: scores^T for both heads land in one 2-bank PSUM
    tile (double-buffered), one batched exp (ACT) per k-tile, softmax
    denominator accumulated per-head on DVE/Pool, PV accumulated in PSUM
    with the PV matmul one k-step behind the score matmul so PE never
    waits on ACT.
  - Attention outputs are AllGathered across the 8 cores per 512-token
    chunk (bf16); the output projection for chunk i-1 is interleaved into
    block i's attention t-loop to fill PE slack, remainder drains in a
    dense tail.  Each core computes its 512-column slice of out.
"""

import math
import os
import sys

if "/opt/trn_rl_repo" not in sys.path:
    sys.path.insert(0, "/opt/trn_rl_repo")

import ml_dtypes
import numpy as np

import concourse.bass as bass
import concourse.mybir as mybir
import concourse.tile as tile
from concourse import bacc
from concourse import bass_utils

# ---- problem constants (hardcoded per harness contract) ----
HIDDEN = 4096
N_HEADS = 32
N_KV_HEADS = 8
HEAD_DIM = 128
ROPE_THETA = 10000.0
B, S = 2, 2048
NCORES = 8

P = 128
TQ = 512                      # token block
NB = S // TQ                  # 4 blocks per batch
KT = HIDDEN // P              # 32 contraction tiles
KB = 8                        # k-tiles per x DMA batch
QH = N_HEADS // NCORES        # 4 q-heads per core
HG = QH * HEAD_DIM            # 512 = head-group width per core
NCHUNK = B * NB               # 8 allgather chunks
TOK = B * S                   # 4096 tokens
TO = 512                      # outproj token sub-block

f32 = mybir.dt.float32
bf16 = mybir.dt.bfloat16
npbf16 = ml_dtypes.bfloat16


def _build_module(with_collectives=True):
    nc = bacc.Bacc("TRN2", target_bir_lowering=False, debug=False,
                   num_devices=NCORES)
    nc._skip_collectives = not with_collectives

    xT = nc.dram_tensor("xT", [HIDDEN, TOK], bf16, kind="ExternalInput").ap()
    wqkvo = nc.dram_tensor("wqkvo", [HIDDEN, 2 * HG + 2 * HEAD_DIM], bf16,
                           kind="ExternalInput").ap()
    tables = nc.dram_tensor("tables", [P, 2 * S + 4 * TQ + 1], bf16,
                            kind="ExternalInput").ap()
    wqT = wqkvo[:, 0:HG]
    wkT = wqkvo[:, HG:HG + HEAD_DIM]
    wvT = wqkvo[:, HG + HEAD_DIM:HG + 2 * HEAD_DIM]
    woT = wqkvo[:, HG + 2 * HEAD_DIM:]
    cosT = tables[:, 0:S]
    ssinT = tables[:, S:2 * S]
    masksIn = tables[:, 2 * S:2 * S + 4 * TQ]
    onesIn = tables[:, 2 * S + 4 * TQ:]
    outT = nc.dram_tensor("outT", [HG, TOK], f32, kind="ExternalOutput").ap()

    ag_in = [
        nc.dram_tensor(f"ag_in{i}", [HG, TQ], bf16, kind="Internal").ap()
        for i in range(NCHUNK)
    ]
    ag_out = [
        nc.dram_tensor(f"ag_out{i}", [HIDDEN, TQ], bf16, kind="Internal",
                       addr_space="Shared").ap()
        for i in range(NCHUNK)
    ]

    with tile.TileContext(nc) as tc:
        _body(tc, nc, xT, wqT, wkT, wvT, woT, cosT, ssinT, masksIn,
              onesIn, outT, ag_in, ag_out)
    nc.compile()
    return nc


class OutprojEmitter:
    """Queue of output-projection emission closures, drained op-by-op into
    PE slack inside the attention t-loops (remainder drains densely)."""

    def __init__(self, nc, wo_sb, atpool, obpool, psum, outT, ag_out):
        self.nc = nc
        self.wo_sb = wo_sb
        self.atpool = atpool
        self.obpool = obpool
        self.psum = psum
        self.outT = outT
        self.ag_out = ag_out
        self.q = []
        self.ob_flip = 0

    def add_chunk(self, ch):
        nc = self.nc
        KC = 8  # matmuls per emitted closure
        ats = {}

        def load(half):
            at = self.atpool.tile([P, KT, TO], bf16, tag="at", bufs=1,
                                  name="at")
            nc.sync.dma_start(
                at[:],
                self.ag_out[ch].rearrange("(ko p) t -> p ko t", p=P)[
                    :, :, half * TO:(half + 1) * TO],
            )
            ats[half] = at

        for half in range(TQ // TO):
            self.q.append(lambda half=half: load(half))
        for half in range(TQ // TO):
            for m in range(HG // P):
                def start_chain(half=half, m=m):
                    self._op_ps = self.psum.tile([P, TO], f32, tag="pop",
                                                 bufs=2, name="op_ps")

                def mms(half=half, m=m, k0=0, first=False, last=False):
                    if first:
                        start_chain(half, m)
                    op_ps, at = self._op_ps, ats[half]
                    for k in range(k0, k0 + KC):
                        nc.tensor.matmul(
                            op_ps[:], self.wo_sb[:, k, m * P:(m + 1) * P],
                            at[:, k, :],
                            start=(k == 0), stop=(k == KT - 1),
                        )
                    if last:
                        ob = self.obpool.tile([P, TO], f32, tag="ob", bufs=2,
                                              name="ob")
                        nc.vector.tensor_copy(ob[:], op_ps[:])
                        c0 = ch * TQ + half * TO
                        nc.sync.dma_start(
                            self.outT[m * P:(m + 1) * P, c0:c0 + TO], ob[:])

                for k0 in range(0, KT, KC):
                    self.q.append(
                        lambda half=half, m=m, k0=k0: mms(
                            half, m, k0, first=(k0 == 0),
                            last=(k0 + KC == KT)))

    def emit(self, n):
        while n > 0 and self.q:
            self.q.pop(0)()
            n -= 1

    def drain(self):
        self.emit(len(self.q))


def _body(tc, nc, xT, wqT, wkT, wvT, woT, cosT, ssinT, masksIn,
          onesIn, outT, ag_in, ag_out):
    AF = mybir.ActivationFunctionType
    OP = mybir.AluOpType

    with (
        tc.tile_pool(name="wpool", bufs=1) as wpool,
        tc.tile_pool(name="xpool", bufs=2) as xpool,
        tc.tile_pool(name="kvpool", bufs=1) as kvpool,
        tc.tile_pool(name="qpool", bufs=1) as qpool,
        tc.tile_pool(name="epool", bufs=3) as epool,
        tc.tile_pool(name="accpool", bufs=1) as accpool,
        tc.tile_pool(name="aux", bufs=2) as aux,
        tc.tile_pool(name="atpool", bufs=2) as atpool,
        tc.tile_pool(name="obpool", bufs=3) as obpool,
        tc.tile_pool(name="psum", bufs=1, space="PSUM") as psum,
    ):
        # ---- resident constants / weights.  QKV weights + x go on the SP
        # DMA queue interleaved (block 0 starts after ~2MB); everything not
        # needed immediately goes on the ACT DMA queue in parallel.
        wq_sb = wpool.tile([P, KT, HG], bf16, tag="wq")
        wk_sb = wpool.tile([P, KT, HEAD_DIM], bf16, tag="wk")
        wv_sb = wpool.tile([P, KT, HEAD_DIM], bf16, tag="wv")

        def load_wq_chunk(c4):
            nc.sync.dma_start(
                wq_sb[:, c4 * 4:(c4 + 1) * 4, :],
                wqT[c4 * 4 * P:(c4 + 1) * 4 * P, :].rearrange(
                    "(ko p) m -> p ko m", p=P))

        load_wq_chunk(0)
        # block-0 x batches are interleaved with the remaining wq chunks and
        # wk/wv inside the first QKV loop (deferred_w), so PE starts after
        # ~1.5MB of DMA instead of the full weight set.
        deferred_w = [
            lambda: nc.sync.dma_start(
                wk_sb[:], wkT.rearrange("(ko p) m -> p ko m", p=P)),
            lambda: nc.sync.dma_start(
                wv_sb[:], wvT.rearrange("(ko p) m -> p ko m", p=P)),
        ] + [lambda c4=c4: load_wq_chunk(c4) for c4 in range(1, 8)]
        cos_sb = wpool.tile([P, S], bf16, tag="cos")
        nc.scalar.dma_start(cos_sb[:], cosT)
        sin_sb = wpool.tile([P, S], bf16, tag="sin")
        nc.scalar.dma_start(sin_sb[:], ssinT)
        mask_sb = wpool.tile([P, 4, TQ], bf16, tag="mask")
        nc.scalar.dma_start(mask_sb[:], masksIn.rearrange("p (r t) -> p r t", r=4))
        ones_sb = wpool.tile([P, 1], bf16, tag="ones")
        nc.scalar.dma_start(ones_sb[:], onesIn)
        wo_sb = wpool.tile([P, KT, HG], bf16, tag="wo")
        nc.scalar.dma_start(wo_sb[:], woT.rearrange("(ko p) m -> p ko m", p=P))

        emitter = OutprojEmitter(nc, wo_sb, atpool, obpool, psum, outT, ag_out)

        def rope(eng, tmptag, dst, n):
            """In-place RoPE on bf16 SBUF tile dst [P, TQ] for block n."""
            cos_blk = cos_sb[:, n * TQ:(n + 1) * TQ]
            sin_blk = sin_sb[:, n * TQ:(n + 1) * TQ]
            rt = aux.tile([P, TQ], bf16, tag=tmptag, bufs=2, name="ropetmp")
            eng.tensor_copy(rt[0:64, :], dst[64:128, :])
            eng.tensor_copy(rt[64:128, :], dst[0:64, :])
            eng.tensor_tensor(rt[:], rt[:], sin_blk, OP.mult)
            eng.tensor_tensor(dst, dst, cos_blk, OP.mult)
            eng.tensor_tensor(dst, dst, rt[:], OP.add)

        def normalize(h, pv_ps, acc, ch):
            """softmax denominator + divide for head h, ship to ag_in."""
            dn_ps = psum.tile([1, TQ], f32, tag="pop", bufs=2, name="dn_ps")
            nc.tensor.matmul(dn_ps[:1, :], ones_sb[:], acc[:],
                             start=True, stop=True)
            rec = aux.tile([1, TQ], f32, tag="rec", name="rec")
            nc.vector.reciprocal(rec[:], dn_ps[:1, :])
            bc = aux.tile([P, TQ], f32, tag="bc", name="bc")
            nc.gpsimd.partition_broadcast(bc[:], rec[:])
            ao = aux.tile([P, TQ], bf16, tag="ao", name="ao")
            nc.vector.tensor_tensor(ao[:], pv_ps, bc[:], OP.mult)
            nc.sync.dma_start(ag_in[ch][h * P:(h + 1) * P, :], ao[:])

        for b in range(B):
            kT_cache = kvpool.tile([P, S], bf16, tag="kT")
            v_cache = kvpool.tile([P, S // P, HEAD_DIM], bf16, tag="v")
            for n in range(NB):
                i_blk = b * NB + n
                ch = i_blk
                tok0 = b * S + n * TQ
                ntk = (n + 1) * (TQ // P)

                # ---------- QKV projection for this token block ----------
                # The whole x block stays resident (two 16-ktile tiles), so
                # the six output chains run sequentially (PSUM accumulation
                # groups are per-bank) and each chain's PSUM drain + RoPE
                # overlaps the later chains.  By the time the k/v chains
                # finish, q0/q1 are already roped, so pass A starts with no
                # boundary stall.
                xa = xpool.tile([P, KT // 2, TQ], bf16, tag="xa", bufs=1, name="xa")
                xb = xpool.tile([P, KT // 2, TQ], bf16, tag="xb", bufs=1, name="xb")
                for xi, xt_ in ((0, xa), (1, xb)):
                    for hh in range(2):
                        ko0 = xi * 16 + hh * 8
                        nc.sync.dma_start(
                            xt_[:, hh * 8:(hh + 1) * 8, :],
                            xT[ko0 * P:(ko0 + 8) * P,
                               tok0:tok0 + TQ].rearrange(
                                "(ko p) t -> p ko t", p=P),
                        )
                        if i_blk == 0:
                            for _ in range(3 if xi == 0 and hh == 0 else 2):
                                if deferred_w:
                                    deferred_w.pop(0)()
                if i_blk >= 1:
                    # at-tile loads for the chunk whose outproj is
                    # interleaved into this block (its AllGather was issued
                    # at the end of the previous block)
                    emitter.add_chunk(i_blk - 1)
                    emitter.emit(1)  # the at-load DMA

                def xk(k):
                    return (xa if k < 16 else xb)[:, k % 16, :]

                qT_sb = qpool.tile([P, QH, TQ], bf16, tag="q", name="qT_sb")
                kblk = kT_cache[:, n * TQ:(n + 1) * TQ]

                q01 = psum.tile([P, 2, TQ], f32, tag="qA", name="q01")
                for j in range(2):
                    for k in range(KT):
                        nc.tensor.matmul(
                            q01[:, j, :], wq_sb[:, k, j * P:(j + 1) * P],
                            xk(k), start=(k == 0), stop=(k == KT - 1))
                nc.scalar.copy(qT_sb[:, 0:2, :], q01[:])
                rope(nc.vector, "rtD", qT_sb[:, 0, :], n)
                rope(nc.vector, "rtD", qT_sb[:, 1, :], n)

                q23 = psum.tile([P, 2, TQ], f32, tag="qB", name="q23")
                for j in range(2):
                    for k in range(KT):
                        nc.tensor.matmul(
                            q23[:, j, :], wq_sb[:, k, (j + 2) * P:(j + 3) * P],
                            xk(k), start=(k == 0), stop=(k == KT - 1))
                nc.scalar.copy(qT_sb[:, 2:4, :], q23[:])
                rope(nc.vector, "rtD", qT_sb[:, 2, :], n)
                rope(nc.vector, "rtD", qT_sb[:, 3, :], n)

                k_ps = psum.tile([P, TQ], f32, tag="kk", name="k_ps")
                for k in range(KT):
                    nc.tensor.matmul(k_ps[:], wk_sb[:, k, :], xk(k),
                                     start=(k == 0), stop=(k == KT - 1))
                nc.scalar.copy(kblk, k_ps[:])
                rope(nc.vector, "rtD", kblk, n)

                v4 = psum.tile([P, 4, HEAD_DIM], f32, tag="vv", name="v4")
                for j in range(4):
                    for k in range(KT):
                        nc.tensor.matmul(
                            v4[:, j, :], xk(k)[:, j * P:(j + 1) * P],
                            wv_sb[:, k, :], start=(k == 0), stop=(k == KT - 1))
                nc.scalar.copy(
                    v_cache[:, n * 4:(n + 1) * 4, :], v4[:])

                # ---------- attention: two heads per pass ----------
                for pas in range(2):
                    h0, h1 = 2 * pas, 2 * pas + 1
                    pv0 = psum.tile([P, TQ], f32, tag="kk", name="pv0")
                    pv1 = psum.tile([P, TQ], f32, tag="vv", name="pv1")
                    acc0 = accpool.tile([P, TQ], bf16, tag=f"acc{h0}",
                                        name="acc0")
                    acc1 = accpool.tile([P, TQ], bf16, tag=f"acc{h1}",
                                        name="acc1")
                    es_prev = None
                    for t in range(ntk):
                        emitter.emit(1)
                        st2 = psum.tile([P, 2, TQ], f32,
                                        tag=("qA" if t % 2 == 0 else "qB"),
                                        name="st2")
                        kt = kT_cache[:, t * P:(t + 1) * P]
                        nc.tensor.matmul(st2[:, 0, :], kt, qT_sb[:, h0, :],
                                         start=True, stop=True)
                        nc.tensor.matmul(st2[:, 1, :], kt, qT_sb[:, h1, :],
                                         start=True, stop=True)
                        if es_prev is not None:
                            tp = t - 1
                            nc.tensor.matmul(
                                pv0[:], v_cache[:, tp, :], es_prev[:, 0, :],
                                start=(tp == 0), stop=False)
                            nc.tensor.matmul(
                                pv1[:], v_cache[:, tp, :], es_prev[:, 1, :],
                                start=(tp == 0), stop=False)
                        es2 = epool.tile([P, 2, TQ], bf16, tag="es",
                                         name="es2")
                        nc.scalar.activation(
                            es2.rearrange("p h t -> p (h t)"),
                            st2.rearrange("p h t -> p (h t)"), AF.Exp)
                        r = t - (ntk - 4)
                        if r >= 0:
                            mk = mask_sb[:, r, :]
                            nc.vector.tensor_tensor(
                                es2[:, 0, :], es2[:, 0, :], mk, OP.mult)
                            nc.vector.tensor_tensor(
                                es2[:, 1, :], es2[:, 1, :], mk, OP.mult)
                        if t == 0:
                            nc.vector.tensor_copy(acc0[:], es2[:, 0, :])
                            nc.gpsimd.tensor_copy(acc1[:], es2[:, 1, :])
                        else:
                            nc.vector.tensor_tensor(
                                acc0[:], acc0[:], es2[:, 0, :], OP.add)
                            nc.gpsimd.tensor_tensor(
                                acc1[:], acc1[:], es2[:, 1, :], OP.add)
                        es_prev = es2
                    tp = ntk - 1
                    nc.tensor.matmul(pv0[:], v_cache[:, tp, :],
                                     es_prev[:, 0, :],
                                     start=(tp == 0), stop=True)
                    nc.tensor.matmul(pv1[:], v_cache[:, tp, :],
                                     es_prev[:, 1, :],
                                     start=(tp == 0), stop=True)
                    normalize(h0, pv0[:], acc0, ch)
                    normalize(h1, pv1[:], acc1, ch)

                # ---------- AllGather this chunk across the 8 cores ----
                if not getattr(nc, "_skip_collectives", False):
                    nc.gpsimd.collective_compute(
                        "AllGather",
                        mybir.AluOpType.bypass,
                        replica_groups=[list(range(NCORES))],
                        ins=[ag_in[ch].opt()],
                        outs=[ag_out[ch].opt()],
                    )

        # ---------- tail: remaining output projection ----------
        emitter.add_chunk(NCHUNK - 1)
        emitter.drain()


_NC_CACHE = None


def _get_module():
    global _NC_CACHE
    if _NC_CACHE is None:
        _NC_CACHE = _build_module(
            with_collectives=not bool(int(os.environ.get("KERNEL_NO_CC", "0"))))
    return _NC_CACHE


def _host_consts():
    inv_freq = 1.0 / (ROPE_THETA ** (np.arange(0, HEAD_DIM, 2,
                                               dtype=np.float32) / HEAD_DIM))
    t = np.arange(S, dtype=np.float32)
    freqs = np.outer(t, inv_freq).astype(np.float32)      # [S, 64]
    cos_h = np.cos(freqs).T                               # [64, S]
    sin_h = np.sin(freqs).T
    cosT = np.concatenate([cos_h, cos_h], axis=0)
    ssinT = np.concatenate([-sin_h, sin_h], axis=0)

    i = np.arange(P)[:, None]
    j = np.arange(TQ)[None, :]
    masks = np.concatenate(
        [(i + r * P <= j).astype(np.float32) for r in range(4)], axis=1
    )                                                     # [128, 4*512]
    ones = np.ones((P, 1), dtype=np.float32)
    return (cosT.astype(npbf16), ssinT.astype(npbf16),
            masks.astype(npbf16), ones.astype(npbf16))


def make_in_maps(hidden_states, wq, wk, wv, wo):
    hidden_states = np.asarray(hidden_states, dtype=np.float32)
    wq = np.asarray(wq, dtype=np.float32)
    wk = np.asarray(wk, dtype=np.float32)
    wv = np.asarray(wv, dtype=np.float32)
    wo = np.asarray(wo, dtype=np.float32)

    xT = np.ascontiguousarray(
        hidden_states.reshape(TOK, HIDDEN).T).astype(npbf16)
    cosT, ssinT, masks, ones = _host_consts()
    qscale = 1.0 / math.sqrt(HEAD_DIM)
    tables = np.concatenate(
        [cosT, ssinT, masks, ones], axis=1).astype(npbf16)

    in_maps = []
    for c in range(NCORES):
        wqkvo = np.concatenate([
            (wq[c * HG:(c + 1) * HG] * qscale).T,
            wk[c * HEAD_DIM:(c + 1) * HEAD_DIM].T,
            wv[c * HEAD_DIM:(c + 1) * HEAD_DIM].T,
            wo[c * HG:(c + 1) * HG].T,
        ], axis=1).astype(npbf16)
        in_maps.append({
            "xT": xT,
            "wqkvo": np.ascontiguousarray(wqkvo),
            "tables": tables,
        })
    return in_maps


def assemble_output(results):
    out = np.empty((TOK, HIDDEN), dtype=np.float32)
    for c in range(NCORES):
        out[:, c * HG:(c + 1) * HG] = results[c]["outT"].T
    return out.reshape(B, S, HIDDEN)


def kernel(hidden_states, wq, wk, wv, wo):
    nc = _get_module()
    in_maps = make_in_maps(hidden_states, wq, wk, wv, wo)
    trace = bool(int(os.environ.get("KERNEL_TRACE", "0")))
    res = bass_utils.run_bass_kernel_spmd(
        nc, in_maps, core_ids=list(range(NCORES)), trace=trace
    )
    if trace:
        kernel.last_results = res
    return assemble_output(res.results)


kernel.last_results = None
